# revision 11
# baseline (speedup 1.0000x reference)
"""DRMM histogram-binning kernel for 8 Trainium2 NeuronCores.

Strategy (data-parallel over batch, 4 batches/core):
  * Host pre-normalizes the embedding table (fp16) and augments each padded
    row with the row norm (slot 300 of a 384-elem row).  Two zero sentinel
    rows are inserted so that int16 gather indices can address the full
    50000-row vocab in two base-offset ranges (lo: rows 0..32767, hi: rows
    32768..50001) with index 0 of each range pointing at a zero row.
  * On device each core transpose-mode dma_gathers its document rows into
    K-major [128, 3, P] fp16 tiles (P positions per batch: sorted lo ids,
    sentinel pad, then hi ids, sentinel pad).  Query rows are gathered the
    same way (lo + hi overlay added together).
  * cosine sim == plain dot product of pre-normalized rows; computed on the
    tensor engine as [128 doc-positions, 16 queries] PSUM tiles.
  * Histogram via threshold counting: one DVE is_ge against the 29 interior
    bin edges gives a [128, 16*29] one-hot-cumulative tile; a matmul with
    the per-position validity mask as the 1-column stationary operand
    reduces over positions into cumulative counts C[1, 464].  Bin counts
    are second differences of C, realized with a replicate matmul plus a
    per-partition select/diff multiply-reduce, giving h[30, 16].
  * log1p + query_len mask + the tiny 30-5-1-1 tanh MLP + softmax gate run
    on chip per batch; each core writes its 4 scalars.
"""

import math
import os
import sys

import numpy as np

B, Q, D, E, V, NBINS = 32, 16, 2048, 300, 50000, 30
NCORES = 8
BPC = B // NCORES            # batches per core
EPAD = 384                   # padded row length (fp16) -> 768 B, %256 == 0
NORM_SLOT = 320              # norm slot; chunk 2, partition 64 (32-aligned)
LO_ROWS = 32768              # rows addressable by the lo-range gather
TROWS = V + 2                # table rows incl. two zero sentinels
QPOS = 128                   # query gather positions (64 real + pad)


def _wrap_idx(idx: np.ndarray, num_idxs: int) -> np.ndarray:
    """Host layout for dma_gather idxs: [128, num_idxs//16] int16 with index
    i at [i % 16, i // 16], replicated across the eight 16-partition groups."""
    assert num_idxs % 16 == 0 and len(idx) == num_idxs
    a = np.zeros((16, num_idxs // 16), np.int16)
    a[:, :] = idx.astype(np.int16).reshape(num_idxs // 16, 16).T
    return np.tile(a, (8, 1))


def _prep_table(emb: np.ndarray) -> np.ndarray:
    norm = np.linalg.norm(emb.astype(np.float32), axis=1)
    nemb = emb.astype(np.float32) / np.maximum(norm, 1e-30)[:, None]
    t = np.zeros((TROWS, EPAD), np.float16)
    t[1 : LO_ROWS, :E] = nemb[: LO_ROWS - 1]
    t[1 : LO_ROWS, NORM_SLOT] = norm[: LO_ROWS - 1]
    t[LO_ROWS + 1 :, :E] = nemb[LO_ROWS - 1 :]
    t[LO_ROWS + 1 :, NORM_SLOT] = norm[LO_ROWS - 1 :]
    return t


def _split_ids(v: np.ndarray) -> tuple[np.ndarray, np.ndarray]:
    """Map vocab ids to (lo-range idx, hi-range idx) lists (sorted)."""
    v = np.sort(v.astype(np.int64))
    lo = v[v <= LO_ROWS - 2] + 1                    # rows 1..32767
    hi = v[v >= LO_ROWS - 1] + 2 - LO_ROWS          # rows 1..17233 of hi view
    return lo, hi


def _build_program(NA: int, NB: int):
    import concourse.bass as bass
    import concourse.mybir as mybir
    import concourse.tile as tile
    from concourse import bacc

    PA, PB = NA * 128, NB * 128
    P = PA + PB
    NCH = NA + NB
    f32, f16, i16 = mybir.dt.float32, mybir.dt.float16, mybir.dt.int16
    AF = mybir.ActivationFunctionType
    OP = mybir.AluOpType

    nc = bacc.Bacc("TRN2", target_bir_lowering=False, debug=False)

    table = nc.dram_tensor("table", [TROWS, EPAD], f16, kind="ExternalInput")
    d_qlo = nc.dram_tensor("qidx_lo", [128, QPOS // 16], i16, kind="ExternalInput")
    d_qhi = nc.dram_tensor("qidx_hi", [128, QPOS // 16], i16, kind="ExternalInput")
    d_dlo = nc.dram_tensor("didx_lo", [128, BPC * PA // 16], i16, kind="ExternalInput")
    d_dhi = nc.dram_tensor("didx_hi", [128, BPC * PB // 16], i16, kind="ExternalInput")
    d_edges = nc.dram_tensor("edges", [128, 16 * 29], f32, kind="ExternalInput")
    d_sel = nc.dram_tensor("selmat", [30, 29], f32, kind="ExternalInput")
    d_h0 = nc.dram_tensor("h0bias", [30, 1], f32, kind="ExternalInput")
    d_mask = nc.dram_tensor("maskrep", [30, BPC * 16], f32, kind="ExternalInput")
    d_w1 = nc.dram_tensor("w1", [30, 5], f32, kind="ExternalInput")
    d_b1 = nc.dram_tensor("b1", [5, 1], f32, kind="ExternalInput")
    d_w2 = nc.dram_tensor("w2", [5, 1], f32, kind="ExternalInput")
    d_b2 = nc.dram_tensor("b2", [1, 1], f32, kind="ExternalInput")
    d_w3 = nc.dram_tensor("w3", [1, 1], f32, kind="ExternalInput")
    d_b3 = nc.dram_tensor("b3", [1, 1], f32, kind="ExternalInput")
    d_gw = nc.dram_tensor("gw32", [128, 3], f32, kind="ExternalInput")
    d_gb = nc.dram_tensor("gbias", [1, 1], f32, kind="ExternalInput")
    d_out = nc.dram_tensor("out", [1, BPC], f32, kind="ExternalOutput")

    tlo = table.ap()[0:LO_ROWS, :]
    thi = table.ap()[LO_ROWS:TROWS, :]

    with tile.TileContext(nc) as tc:
        with (
            tc.tile_pool(name="const", bufs=1) as cpool,
            tc.tile_pool(name="de", bufs=2) as depool,
            tc.tile_pool(name="sim", bufs=4) as simpool,
            tc.tile_pool(name="oh", bufs=4) as ohpool,
            tc.tile_pool(name="small", bufs=2) as spool,
            tc.tile_pool(name="ps_sim", bufs=2, space="PSUM") as ps_sim,
            tc.tile_pool(name="ps_rep", bufs=2, space="PSUM") as ps_rep,
            tc.tile_pool(name="ps_mlp", bufs=2, space="PSUM") as ps_mlp,
            tc.tile_pool(name="ps_gate", bufs=1, space="PSUM") as ps_gate,
        ):
            def load(dram, shape, dtype, tag):
                t = cpool.tile(shape, dtype, tag=tag)
                nc.sync.dma_start(out=t[:], in_=dram.ap())
                return t

            qlo_i = load(d_qlo, [128, QPOS // 16], i16, tag='qlo_i')
            qhi_i = load(d_qhi, [128, QPOS // 16], i16, tag='qhi_i')
            dlo_i = load(d_dlo, [128, BPC * PA // 16], i16, tag='dlo_i')
            dhi_i = load(d_dhi, [128, BPC * PB // 16], i16, tag='dhi_i')
            edges = load(d_edges, [128, 16 * 29], f32, tag='edges')
            sel = load(d_sel, [30, 29], f32, tag='sel')
            h0b = load(d_h0, [30, 1], f32, tag='h0b')
            mrep = load(d_mask, [30, BPC * 16], f32, tag='mrep')
            w1 = load(d_w1, [30, 5], f32, tag='w1')
            b1 = load(d_b1, [5, 1], f32, tag='b1')
            w2 = load(d_w2, [5, 1], f32, tag='w2')
            b2 = load(d_b2, [1, 1], f32, tag='b2')
            w3 = load(d_w3, [1, 1], f32, tag='w3')
            b3 = load(d_b3, [1, 1], f32, tag='b3')
            gw = load(d_gw, [128, 3], f32, tag='gw')
            gbi = load(d_gb, [1, 1], f32, tag='gbi')
            ones30 = cpool.tile([128, 30], f32, tag='ones30')
            nc.vector.memset(ones30[:], 1.0)

            # ---- query rows: lo + hi overlay ----
            qt_lo = cpool.tile([128, 3, QPOS], f16, tag='qt_lo')
            qt_hi = cpool.tile([128, 3, QPOS], f16, tag='qt_hi')
            nc.gpsimd.dma_gather(qt_lo[:], tlo, qlo_i[:], QPOS, QPOS, EPAD,
                                 transpose=True, single_packet=False)
            nc.gpsimd.dma_gather(qt_hi[:], thi, qhi_i[:], QPOS, QPOS, EPAD,
                                 transpose=True, single_packet=False)
            qt = cpool.tile([128, 3, QPOS], f16, tag='qt')
            nc.vector.tensor_tensor(out=qt[:], in0=qt_lo[:], in1=qt_hi[:],
                                    op=OP.add)
            # query norms live at elem 320 -> chunk 2, partition 64
            qn = cpool.tile([1, QPOS], f32, tag='qn')
            nc.vector.tensor_copy(out=qn[:], in_=qt[64:65, 2, :])
            nc.vector.memset(qt[64:65, 2, :], 0.0)
            qt32 = cpool.tile([128, 3, QPOS], f32, tag='qt32')
            nc.vector.tensor_copy(out=qt32[:], in_=qt[:])

            # ---- gate: softmax(qn * (nqe @ gw) + gb) over q ----
            gacc = ps_gate.tile([1, QPOS], f32)
            for c in range(3):
                nc.tensor.matmul(out=gacc[:], lhsT=gw[:, c : c + 1],
                                 rhs=qt32[:, c, :], start=(c == 0), stop=(c == 2))
            glog = cpool.tile([1, QPOS], f32, tag='glog')
            nc.vector.tensor_tensor(out=glog[:], in0=gacc[:], in1=qn[:],
                                    op=OP.mult)
            esb = cpool.tile([1, QPOS], f32, tag='esb')
            nc.scalar.activation(esb[:], glog[:], AF.Exp, bias=gbi[:, 0:1])
            s4 = cpool.tile([1, BPC], f32, tag='s4')
            nc.vector.tensor_reduce(
                out=s4[:],
                in_=esb[:, 0 : BPC * 16].rearrange("p (b q) -> p b q", q=16),
                axis=mybir.AxisListType.X, op=OP.add)
            rs4 = cpool.tile([1, BPC], f32, tag='rs4')
            nc.vector.reciprocal(out=rs4[:], in_=s4[:])

            zall = cpool.tile([1, BPC * 16], f32, tag='zall')

            for b in range(BPC):
                de_a = depool.tile([128, 3, PA], f16, tag="dea")
                de_b = depool.tile([128, 3, PB], f16, tag="deb")
                nc.gpsimd.dma_gather(
                    de_a[:], tlo,
                    dlo_i[:, b * (PA // 16) : (b + 1) * (PA // 16)],
                    PA, PA, EPAD, transpose=True, single_packet=False)
                nc.gpsimd.dma_gather(
                    de_b[:], thi,
                    dhi_i[:, b * (PB // 16) : (b + 1) * (PB // 16)],
                    PB, PB, EPAD, transpose=True, single_packet=False)

                rep = ps_rep.tile([30, 16 * 29], mybir.dt.float32, tag="rep")
                for pc in range(NCH):
                    if pc < NA:
                        de_sl = lambda c, pc=pc: de_a[:, c, pc * 128 : (pc + 1) * 128]
                    else:
                        de_sl = lambda c, pc=pc: de_b[:, c, (pc - NA) * 128 : (pc - NA + 1) * 128]
                    sim_ps = ps_sim.tile([128, 16], mybir.dt.float32, tag="s")
                    for c in range(3):
                        nc.tensor.matmul(
                            out=sim_ps[:],
                            lhsT=de_sl(c),
                            rhs=qt[:, c, b * 16 : (b + 1) * 16],
                            start=(c == 0), stop=(c == 2))
                    sim32 = simpool.tile([128, 16], f32, tag="sim32")
                    nc.scalar.activation(sim32[:], sim_ps[:], AF.Copy)
                    oh = ohpool.tile([128, 16 * 29], f32, tag="oh")
                    nc.vector.tensor_tensor(
                        out=oh[:].rearrange("p (q t) -> p q t", t=29),
                        in0=sim32[:].rearrange("p (q o) -> p q o", o=1)
                            .to_broadcast([128, 16, 29]),
                        in1=edges[:].rearrange("p (q t) -> p q t", t=29),
                        op=OP.is_ge)
                    nc.tensor.matmul(
                        out=rep[:], lhsT=ones30[:],
                        rhs=oh[:], start=(pc == 0), stop=(pc == NCH - 1))

                tmp = spool.tile([30, 16 * 29], f32, tag="tmp")
                nc.vector.tensor_tensor(
                    out=tmp[:].rearrange("p (q t) -> p q t", t=29),
                    in0=rep[:].rearrange("p (q t) -> p q t", t=29),
                    in1=sel[:].rearrange("p (o t) -> p o t", o=1)
                        .to_broadcast([30, 16, 29]),
                    op=OP.mult)
                h2 = spool.tile([30, 16], f32, tag="h2")
                nc.vector.tensor_reduce(
                    out=h2[:], in_=tmp[:].rearrange("p (q t) -> p q t", t=29),
                    axis=mybir.AxisListType.X, op=OP.add)
                nc.vector.tensor_scalar_add(h2[:], h2[:], h0b[:, 0:1])
                ll = spool.tile([30, 16], f32, tag="ll")
                nc.scalar.activation(ll[:], h2[:], AF.Ln, bias=1.0)
                lm = spool.tile([30, 16], f32, tag="lm")
                nc.vector.tensor_tensor(
                    out=lm[:], in0=ll[:],
                    in1=mrep[:, b * 16 : (b + 1) * 16], op=OP.mult)

                z1p = ps_mlp.tile([5, 16], mybir.dt.float32, tag="z")
                nc.tensor.matmul(out=z1p[:], lhsT=w1[:], rhs=lm[:],
                                 start=True, stop=True)
                z1 = spool.tile([5, 16], f32, tag="z1")
                nc.scalar.activation(z1[:], z1p[:], AF.Tanh, bias=b1[:, 0:1])
                z2p = ps_mlp.tile([1, 16], mybir.dt.float32, tag="z")
                nc.tensor.matmul(out=z2p[:], lhsT=w2[:], rhs=z1[:],
                                 start=True, stop=True)
                z2 = spool.tile([1, 16], f32, tag="z2")
                nc.scalar.activation(z2[:], z2p[:], AF.Tanh, bias=b2[:, 0:1])
                z3p = ps_mlp.tile([1, 16], mybir.dt.float32, tag="z")
                nc.tensor.matmul(out=z3p[:], lhsT=w3[:], rhs=z2[:],
                                 start=True, stop=True)
                nc.scalar.activation(zall[:, b * 16 : (b + 1) * 16], z3p[:],
                                     AF.Tanh, bias=b3[:, 0:1])

            ze = cpool.tile([1, BPC * 16], f32, tag='ze')
            nc.vector.tensor_tensor(out=ze[:], in0=zall[:],
                                    in1=esb[:, 0 : BPC * 16], op=OP.mult)
            t4 = cpool.tile([1, BPC], f32, tag='t4')
            nc.vector.tensor_reduce(
                out=t4[:], in_=ze[:].rearrange("p (b q) -> p b q", q=16),
                axis=mybir.AxisListType.X, op=OP.add)
            y4 = cpool.tile([1, BPC], f32, tag='y4')
            nc.vector.tensor_tensor(out=y4[:], in0=t4[:], in1=rs4[:],
                                    op=OP.mult)
            nc.sync.dma_start(out=d_out.ap(), in_=y4[:])

    nc.compile()
    return nc


def _prep_inputs(query, query_len, document, emb, w1, b1, w2, b2, w3, b3,
                 gw, gb):
    """Host-side prep: returns (NA, NB, in_maps)."""
    query = np.asarray(query).astype(np.int64)
    document = np.asarray(document).astype(np.int64)
    query_len = np.asarray(query_len).astype(np.int64)
    emb = np.asarray(emb, dtype=np.float32)

    table = _prep_table(emb)

    # split sizes across the whole batch decide the uniform program shape
    splits = [_split_ids(document[g]) for g in range(B)]
    NA = max(1, math.ceil(max(len(lo) for lo, _ in splits) / 128))
    NB = max(1, math.ceil(max(len(hi) for _, hi in splits) / 128))
    PA, PB = NA * 128, NB * 128
    NCH = NA + NB

    edges = np.zeros((128, 16 * 29), np.float32)
    ev = np.arange(1, 30, dtype=np.float32) / 15.0 - 1.0
    edges[:, :] = np.tile(ev, 16)[None, :]

    sel = np.zeros((30, 29), np.float32)
    sel[0, 0] = -1.0
    for j in range(1, 29):
        sel[j, j - 1] = 1.0
        sel[j, j] = -1.0
    sel[29, 28] = 1.0
    # junk sentinel rows have sim == 0 exactly -> they inflate C_1..C_15 by
    # J = P-2048 each; h_0 = P - C_1 and h_15 must drop J.
    J = float(PA + PB - D)
    h0 = np.zeros((30, 1), np.float32)
    h0[0, 0] = float(PA + PB)
    h0[15, 0] = -J

    shared = {
        "table": table,
        "edges": edges,
        "selmat": sel,
        "h0bias": h0,
        "w1": np.asarray(w1, np.float32).reshape(30, 5),
        "b1": np.asarray(b1, np.float32).reshape(5, 1),
        "w2": np.asarray(w2, np.float32).reshape(5, 1),
        "b2": np.asarray(b2, np.float32).reshape(1, 1),
        "w3": np.asarray(w3, np.float32).reshape(1, 1),
        "b3": np.asarray(b3, np.float32).reshape(1, 1),
        "gbias": np.asarray(gb, np.float32).reshape(1, 1),
    }
    gw32 = np.zeros((128, 3), np.float32)
    gwf = np.asarray(gw, np.float32).reshape(E)
    for c in range(3):
        n = min(128, E - c * 128)
        if n > 0:
            gw32[:n, c] = gwf[c * 128 : c * 128 + n]
    shared["gw32"] = gw32

    in_maps = []
    for k in range(NCORES):
        gbs = range(k * BPC, (k + 1) * BPC)
        # query indices at fixed positions b*16+q, padded to 128
        qv = np.full(QPOS, -1, np.int64)
        for i, g in enumerate(gbs):
            qv[i * 16 : (i + 1) * 16] = query[g]
        qlo = np.where((qv >= 0) & (qv <= LO_ROWS - 2), qv + 1, 0)
        qhi = np.where(qv >= LO_ROWS - 1, qv + 2 - LO_ROWS, 0)

        dlo = np.zeros(BPC * PA, np.int64)
        dhi = np.zeros(BPC * PB, np.int64)
        mrep = np.zeros((30, BPC * 16), np.float32)
        for i, g in enumerate(gbs):
            lo, hi = splits[g]
            assert len(lo) <= PA and len(hi) <= PB
            dlo[i * PA : i * PA + len(lo)] = lo
            dhi[i * PB : i * PB + len(hi)] = hi
            mrep[:, i * 16 : (i + 1) * 16] = (
                np.arange(16) < query_len[g]).astype(np.float32)[None, :]

        m = dict(shared)
        m["qidx_lo"] = _wrap_idx(qlo, QPOS)
        m["qidx_hi"] = _wrap_idx(qhi, QPOS)
        m["didx_lo"] = np.concatenate(
            [_wrap_idx(dlo[i * PA : (i + 1) * PA], PA) for i in range(BPC)],
            axis=1)
        m["didx_hi"] = np.concatenate(
            [_wrap_idx(dhi[i * PB : (i + 1) * PB], PB) for i in range(BPC)],
            axis=1)
        m["maskrep"] = mrep
        in_maps.append(m)
    return NA, NB, in_maps


def kernel(**inputs) -> np.ndarray:
    from concourse.bass_utils import run_bass_kernel_spmd

    NA, NB, in_maps = _prep_inputs(
        inputs["query"], inputs["query_len"], inputs["document"],
        inputs["emb"], inputs["w1"], inputs["b1"], inputs["w2"], inputs["b2"],
        inputs["w3"], inputs["b3"], inputs["gw"], inputs["gb"])
    nc = _build_program(NA, NB)
    res = run_bass_kernel_spmd(
        nc, in_maps, core_ids=list(range(NCORES)),
        trace=bool(int(os.environ.get("DRMM_TRACE", "0"))))
    out = np.concatenate(
        [res.results[k]["out"].reshape(BPC) for k in range(NCORES)])
    global LAST_EXEC_NS, LAST_RESULTS
    LAST_RESULTS = res
    if getattr(res, "exec_time_ns", None):
        LAST_EXEC_NS = res.exec_time_ns
        print(f"[kernel] exec_time_ns={res.exec_time_ns} "
              f"mean={getattr(res, 'mean_exec_time_ns', None)}",
              file=sys.stderr)
    return out.astype(np.float32)


LAST_EXEC_NS = None
LAST_RESULTS = None
